# revision 28
# baseline (speedup 1.0000x reference)
"""Trainium2 Bass kernel for nn_CrossAttentionLayer.

Computation (per row b of the batch):
  Q = query @ Wq + bq ; K = kv @ Wk + bk ; V = kv @ Wv + bv   (heads H=8, HD=128)
  scores[h,g] = Q[h]·K[g]/sqrt(128); attn = softmax_g; attended[h] = sum_g attn·V[g]
  out = LN(attended @ Wo + bo + query) * gamma + beta

Strategy: pure data parallel over 8 cores (8192 rows each). Per core,
supertiles of 512 rows. Projections run on the TensorEngine in bf16 with
fp32 PSUM accumulation. The per-sample 8x8 head attention is turned into
dense 128x128 matmuls by grouping 16 samples: for a group, rows (s,h)
of a reshaped Q face rows (s,g) of reshaped K; the full 128x128 product
contains the 16 wanted 8x8 blocks on its block diagonal, which a
block-diagonal mask isolates after exp. The resulting block-diagonal
softmax matrix is itself the operand of the attended matmul, so no
per-sample work is ever done on the vector engine.

Layouts:
  XT/YT  [128 d, 8 k, 512 b]  bf16 (DMA-transposed from DRAM bf16 copies)
  QT/KT  [128 d, 8 h, 512 b]  bf16 (matmul: lhsT=W chunk, rhs=XT);
         per-head contiguous writes; score-group operands are multi-dim
         strided AP views (scores row m=(h,s) h-major, col n=(s,g) s-major)
  V16    [128 b, 1024]        bf16 (matmul: lhsT=YT chunk, rhs=Wv)
  VR     [128 (s,g), 8 jj, 128 d]  group reshape of V16 (stream-copy DMA)
  scores psum [128 (h,s), 128 (s,g)] per 16-sample group
  attT   [128 d, 8 h, 128 b]  bf16 -> lhsT views for the O projection
bo is folded into the residual query on the host, so the O projection
needs no PSUM bias seeding.
"""
import numpy as np
import ml_dtypes
from contextlib import ExitStack

import concourse.bass as bass
import concourse.tile as tile
from concourse import mybir
from concourse.bass_utils import run_bass_kernel_spmd

BF16 = ml_dtypes.bfloat16
F32 = mybir.dt.float32
BF = mybir.dt.bfloat16
AF = mybir.ActivationFunctionType
OP = mybir.AluOpType

N_CORES = 8
B, D, H, HD = 65536, 1024, 8, 128
RPC = B // N_CORES          # rows per core
ST = 512                    # supertile rows
GS = 16                     # samples per attention group (GS*H = 128)
EPS = 1e-5
ISQ = float(1.0 / np.sqrt(HD))


def _build(rows: int, epochs: int = 1) -> bass.Bass:
    nc = bass.Bass("TRN2", target_bir_lowering=False, debug=False,
                   enable_partition_id=False)

    def din(name, shape, dt):
        return nc.dram_tensor(name, shape, dt, kind="ExternalInput").ap()

    query = din("query", [rows, D], F32)
    x16d = din("x16", [rows, D], BF)
    y16d = din("y16", [rows, D], BF)
    wq_d = din("wq", [D, D], BF)
    wk_d = din("wk", [D, D], BF)
    wv_d = din("wv", [D, D], BF)
    wo_d = din("wo", [D, D], BF)
    bq_d = din("bq_dh", [HD, H], F32)
    bk_d = din("bk_dh", [HD, H], F32)
    bvb_d = din("bv_b", [128, D], BF)
    gam_d = din("gamma_b", [128, D], BF)
    bet_d = din("beta_b", [128, D], F32)
    mask_d = din("mask_bd", [128, 128], BF)
    id_d = din("ident", [128, 128], BF)
    out_d = nc.dram_tensor("out", [rows, D], F32, kind="ExternalOutput").ap()

    n_st = rows // ST

    with tile.TileContext(nc) as tc, ExitStack() as ctx:
        cpool = ctx.enter_context(tc.tile_pool(name="consts", bufs=1))
        wq = cpool.tile([128, 8, D], BF, tag="wq")
        wk = cpool.tile([128, 8, D], BF, tag="wk")
        wv = cpool.tile([128, 8, D], BF, tag="wv")
        wo = cpool.tile([128, 8, D], BF, tag="wo")
        nc.sync.dma_start(wq[:], wq_d.rearrange("(k p) n -> p k n", p=128))
        nc.sync.dma_start(wk[:], wk_d.rearrange("(k p) n -> p k n", p=128))
        nc.sync.dma_start(wv[:], wv_d.rearrange("(k p) n -> p k n", p=128))
        nc.sync.dma_start(wo[:], wo_d.rearrange("(k p) n -> p k n", p=128))
        bq = cpool.tile([HD, H], F32, tag="bq")
        bk = cpool.tile([HD, H], F32, tag="bk")
        bvb = cpool.tile([128, D], BF, tag="bvb")
        gam = cpool.tile([128, D], BF, tag="gam")
        bet = cpool.tile([128, D], F32, tag="bet")
        mask = cpool.tile([128, 128], BF, tag="mask")
        ident = cpool.tile([128, 128], BF, tag="ident")
        nc.sync.dma_start(bq[:], bq_d)
        nc.sync.dma_start(bk[:], bk_d)
        nc.sync.dma_start(bvb[:], bvb_d)
        nc.sync.dma_start(gam[:], gam_d)
        nc.sync.dma_start(bet[:], bet_d)
        nc.sync.dma_start(mask[:], mask_d)
        nc.sync.dma_start(ident[:], id_d)
        epsc = cpool.tile([128, 1], F32, tag="epsc")
        nc.vector.memset(epsc[:], EPS)

        stp = ctx.enter_context(tc.tile_pool(name="stp", bufs=2))
        tp = ctx.enter_context(tc.tile_pool(name="tp", bufs=2))
        tp4 = ctx.enter_context(tc.tile_pool(name="tp4", bufs=4))
        qp = ctx.enter_context(tc.tile_pool(name="qtkt", bufs=2))
        ps = ctx.enter_context(tc.tile_pool(name="ps", bufs=6, space="PSUM"))
        psb = ctx.enter_context(tc.tile_pool(name="psb", bufs=2, space="PSUM"))


        def load_transposed(st):
            s0 = (st % n_st) * ST
            xt = stp.tile([128, 8, ST], BF, tag="xt")
            yt = stp.tile([128, 8, ST], BF, tag="yt")
            for k in range(8):
                nc.sync.dma_start_transpose(
                    xt[:, k, :], x16d[s0:s0 + ST, k * 128:(k + 1) * 128])
                nc.sync.dma_start_transpose(
                    yt[:, k, :], y16d[s0:s0 + ST, k * 128:(k + 1) * 128])
            return xt, yt

        n_iter = n_st * epochs
        nxt = load_transposed(0)
        for st in range(n_iter):
            s0 = (st % n_st) * ST
            # ---- A) transposed activations (prefetched) ----
            xt, yt = nxt

            # ---- B) QT/KT projections (transposed layout) ----
            # Matmul operands must flatten to ONE free dim, so each
            # 16-sample group's column block must be contiguous in SBUF.
            # qt [d, grp, h, s]: scores rows m=(h,s) h-major; per-head
            # writes are 16-elem (32B) chunks instead of 2B scatter.
            # kt [d, grp, s, g]: scores cols n=(s,g) s-major (must match
            # VR's partition order); per-head writes are 2B scatter, so
            # split them across ACT and DVE to halve each engine's share.
            NG = ST // GS
            qt = qp.tile([128, NG, H, GS], BF, tag="qt")
            kt = qp.tile([128, NG, GS, H], BF, tag="kt")
            for h in range(H):
                p_q = ps.tile([128, 512], F32, tag="ps")
                for k in range(8):
                    nc.tensor.matmul(p_q[:], wq[:, k, h * 128:(h + 1) * 128],
                                     xt[:, k, :], start=(k == 0), stop=(k == 7))
                # bias add + cast to bf16 (ACT, per-partition bias)
                nc.scalar.activation(
                    qt[:, :, h, :], p_q[:].rearrange("p (j s) -> p j s", s=GS),
                    AF.Identity, bias=bq[:, h:h + 1])
                p_k = ps.tile([128, 512], F32, tag="ps")
                for k in range(8):
                    nc.tensor.matmul(p_k[:], wk[:, k, h * 128:(h + 1) * 128],
                                     yt[:, k, :], start=(k == 0), stop=(k == 7))
                ktv = kt[:, :, :, h]
                pkv = p_k[:].rearrange("p (j s) -> p j s", s=GS)
                if h % 2 == 0:
                    nc.scalar.activation(ktv, pkv, AF.Identity,
                                         bias=bk[:, h:h + 1])
                else:
                    nc.vector.tensor_scalar_add(ktv, pkv, bk[:, h:h + 1])

            if st + 1 < n_iter:
                nxt = load_transposed(st + 1)

            # ---- phase-major tile processing: PE never waits for a
            # tile's softmax chain; it moves on to the next tile's matmuls
            # while DVE/ACT work through the previous one.
            v16s, vrs, e16s, ats, atTs, attTs, rss = {}, {}, {}, {}, {}, {}, {}

            for t in range(4):
                b0 = t * 128
                # V natural + group reshape
                v16 = tp.tile([128, D], BF, tag="v16")
                for nh in range(2):
                    p_v = ps.tile([128, 512], F32, tag="ps")
                    for k in range(8):
                        nc.tensor.matmul(p_v[:], yt[:, k, b0:b0 + 128],
                                         wv[:, k, nh * 512:(nh + 1) * 512],
                                         start=(k == 0), stop=(k == 7))
                    nc.vector.tensor_tensor(v16[:, nh * 512:(nh + 1) * 512], p_v[:],
                                            bvb[:, nh * 512:(nh + 1) * 512], op=OP.add)
                vr = tp4.tile([128, 8, 128], BF, tag="vr")
                for jj in range(8):
                    nc.sync.dma_start(vr[:, jj, :], v16[jj * GS:(jj + 1) * GS, :])
                v16s[t], vrs[t] = v16, vr

            for t in range(4):
                b0 = t * 128
                # scores + exp + softmax chain; mask/rowsum run per half
                # so the chain tail is short once the second half's exp
                # lands (transposes need `at` ~3us after the last score)
                e16 = tp.tile([128, 8, 128], BF, tag="e16")
                ssum = tp.tile([128, 8], F32, tag="ssum")
                for half in range(2):
                    p_s = ps.tile([128, 512], F32, tag="ps")
                    for q4 in range(4):
                        jj = half * 4 + q4
                        gi = t * 8 + jj  # global group index in supertile
                        # rows m=(h,s) h-major; cols n=(s,g) s-major (to
                        # match VR's partition order)
                        qtv = qt[:, gi, :, :].rearrange("p h s -> p (h s)")
                        ktv = kt[:, gi, :, :].rearrange("p s g -> p (s g)")
                        nc.tensor.matmul(p_s[:, q4 * 128:(q4 + 1) * 128], qtv, ktv,
                                         start=True, stop=True)
                    hs = slice(half * 4, (half + 1) * 4)
                    ev = e16[:, hs, :].rearrange("p a b -> p (a b)")
                    nc.scalar.activation(ev, p_s[:], AF.Exp, scale=ISQ)
                    nc.vector.tensor_tensor(
                        e16[:, hs, :], e16[:, hs, :],
                        mask[:, None, :].to_broadcast((128, 4, 128)), op=OP.mult)
                    nc.vector.tensor_reduce(ssum[:, hs], e16[:, hs, :],
                                            axis=mybir.AxisListType.X, op=OP.add)
                rs = tp.tile([128, 8], F32, tag="rs")
                nc.vector.reciprocal(rs[:], ssum[:])
                at = tp4.tile([128, 8, 128], BF, tag="at")
                nc.vector.tensor_tensor(
                    at[:], e16[:],
                    rs[:, :, None].to_broadcast((128, 8, 128)), op=OP.mult)
                e16s[t], ats[t], rss[t] = e16, at, rs

            for t in range(4):
                # attn block transposes
                at = ats[t]
                atT = tp4.tile([128, 8, 128], BF, tag="atT")
                for half in range(2):
                    p_t = psb.tile([128, 512], BF, tag="psb")
                    for q4 in range(4):
                        jj = half * 4 + q4
                        nc.tensor.transpose(p_t[:, q4 * 128:(q4 + 1) * 128],
                                            at[:, jj, :], ident[:])
                    av = atT[:, half * 4:(half + 1) * 4, :].rearrange("p a b -> p (a b)")
                    nc.scalar.copy(av, p_t[:])
                atTs[t] = atT

            for t in range(4):
                # attended
                vr, atT = vrs[t], atTs[t]
                attT = tp4.tile([128, H, 128], BF, tag="attT")
                for half in range(2):
                    p_a = ps.tile([128, 512], F32, tag="ps")
                    for q4 in range(4):
                        jj = half * 4 + q4
                        nc.tensor.matmul(p_a[:, q4 * 128:(q4 + 1) * 128],
                                         vr[:, jj, :], atT[:, jj, :],
                                         start=True, stop=True)
                    av = (attT[:, :, half * 64:(half + 1) * 64]
                          .rearrange("p h (q s) -> p q h s", q=4))
                    nc.scalar.copy(av, p_a[:].rearrange("p (q h s) -> p q h s",
                                                        q=4, h=H))
                attTs[t] = attT

            def o_proj(t):
                b0 = t * 128
                g0 = s0 + b0
                attT = attTs[t]
                # O projection + residual (xres reads also drain the PSUM
                # banks the next O matmuls need)
                qch = tp.tile([128, D], F32, tag="qch")
                nc.sync.dma_start(qch[:], query[g0:g0 + 128, :])
                xres = tp.tile([128, D], F32, tag="xres")
                stats = tp.tile([128, 4], F32, tag="stats")  # [xs0,xs1,sq0,sq1]
                for nh in range(2):
                    p_o = ps.tile([128, 512], F32, tag="ps")
                    for h in range(H):
                        nc.tensor.matmul(p_o[:], attT[:, h, :],
                                         wo[:, h, nh * 512:(nh + 1) * 512],
                                         start=(h == 0), stop=(h == H - 1))
                    nc.vector.scalar_tensor_tensor(
                        xres[:, nh * 512:(nh + 1) * 512], p_o[:], 1.0,
                        qch[:, nh * 512:(nh + 1) * 512], op0=OP.mult, op1=OP.add,
                        accum_out=stats[:, nh:nh + 1])
                    # dummy Square output goes to SBUF scratch, not PSUM —
                    # only the accumulator is consumed
                    sq_scr = tp.tile([128, 512], BF, tag="sq_scr")
                    nc.scalar.activation(sq_scr[:], xres[:, nh * 512:(nh + 1) * 512],
                                         AF.Square, accum_out=stats[:, 2 + nh:3 + nh])
                return (g0, xres, stats)

            def ln_tail(args):
                g0, xres, stats = args
                msum = tp.tile([128, 2], F32, tag="msum")
                nc.vector.tensor_reduce(msum[:], stats[:].rearrange(
                    "p (a b) -> p a b", a=2), axis=mybir.AxisListType.X, op=OP.add)
                mu_ex2 = tp.tile([128, 2], F32, tag="mu_ex2")
                nc.vector.tensor_scalar_mul(mu_ex2[:], msum[:], 1.0 / D)
                mu = mu_ex2[:, 0:1]
                mu2 = tp.tile([128, 1], F32, tag="mu2")
                nc.vector.tensor_tensor(mu2[:], mu, mu, op=OP.mult)
                var = tp.tile([128, 1], F32, tag="var")
                nc.vector.tensor_tensor(var[:], mu_ex2[:, 1:2], mu2[:],
                                        op=OP.subtract)
                lnv = tp.tile([128, 1], F32, tag="lnv")
                nc.scalar.activation(lnv[:], var[:], AF.Ln, bias=epsc[:])
                rstd = tp.tile([128, 1], F32, tag="rstd")
                nc.scalar.activation(rstd[:], lnv[:], AF.Exp, scale=-0.5)
                negc = tp.tile([128, 1], F32, tag="negc")
                nc.vector.tensor_scalar(negc[:], rstd[:], mu, -1.0,
                                        op0=OP.mult, op1=OP.mult)

                tn = tp.tile([128, D], F32, tag="tn")
                nc.vector.tensor_scalar(tn[:], xres[:], rstd[:], negc[:],
                                        op0=OP.mult, op1=OP.add)
                nc.gpsimd.tensor_tensor(tn[:], tn[:], gam[:], op=OP.mult)
                nc.gpsimd.tensor_tensor(tn[:], tn[:], bet[:], op=OP.add)
                nc.sync.dma_start(out_d[g0:g0 + 128, :], tn[:])

            # LN chains trail their tile-pair so the DVE queue is clear of
            # LN small-ops when the next tile's O-proj needs its PSUM bank
            # drained (the xres reads are what release the banks)
            for t0 in (0, 2):
                a0 = o_proj(t0)
                a1 = o_proj(t0 + 1)
                ln_tail(a0)
                ln_tail(a1)

    return nc


def _split_sync_waits(nc, cap=1):
    """This container's walrus build rejects instructions carrying more
    than `cap` semaphore waits (CoreV3 setupSyncWait: "Too many sync
    wait commands").  Tile's kernel-tail drain aggregates one wait per
    active processor, so redistribute: move leading waits onto fresh
    same-engine Drain instructions inserted just before the offender."""
    import bass_rust
    n_split = 0
    for fn in nc.m.functions:
        for blk in fn.blocks:
            new_insts = []
            changed = False
            for inst in blk.instructions:
                si = inst.sync_info
                waits = list(si.on_wait) if si is not None else []
                if len(waits) > cap:
                    changed = True
                    head, keep = waits[:-cap], waits[-cap:]
                    for i in range(0, len(head), cap):
                        d = bass_rust.InstDrain(
                            name=f"{inst.name}-wsplit{i}", is_reset_sema=False)
                        d.engine = inst.engine
                        d.sync_info = bass_rust.SyncInfo(
                            on_wait=head[i:i + cap], on_update=[])
                        new_insts.append(d)
                        n_split += 1
                    inst.sync_info.on_wait = keep
                new_insts.append(inst)
            if changed:
                blk.instructions = new_insts
    return n_split


_built = {}


def _get_nc(rows: int) -> bass.Bass:
    if rows not in _built:
        nc = _build(rows)
        _split_sync_waits(nc)
        _built[rows] = nc
    return _built[rows]


def _prep_consts(Wq, bq, Wk, bk, Wv, bv, Wo, bo, gamma, beta):
    # scores mask: rows m=(h,s) h-major, cols n=(s,g) s-major; keep (m,n)
    # iff both index the same sample s of the 16-sample group
    s_of_m = np.arange(128) % GS
    s_of_n = np.arange(128) // H
    mask = (s_of_m[:, None] == s_of_n[None, :]).astype(np.float32)
    return {
        "wq": np.ascontiguousarray(Wq).astype(BF16),
        "wk": np.ascontiguousarray(Wk).astype(BF16),
        "wv": np.ascontiguousarray(Wv).astype(BF16),
        "wo": np.ascontiguousarray(Wo).astype(BF16),
        "bq_dh": np.ascontiguousarray(np.asarray(bq, np.float32).reshape(H, HD).T),
        "bk_dh": np.ascontiguousarray(np.asarray(bk, np.float32).reshape(H, HD).T),
        "bv_b": np.broadcast_to(np.asarray(bv).astype(BF16), (128, D)).copy(),
        "gamma_b": np.broadcast_to(np.asarray(gamma).astype(BF16), (128, D)).copy(),
        "beta_b": np.broadcast_to(np.asarray(beta, np.float32), (128, D)).copy(),
        "mask_bd": mask.astype(BF16),
        "ident": np.eye(128, dtype=np.float32).astype(BF16),
    }


def make_in_maps(query, key_value, Wq, bq, Wk, bk, Wv, bv, Wo, bo, gamma, beta,
                 rows=RPC, n_cores=N_CORES):
    """CPU-side preprocessing + per-core input maps."""
    x16 = np.ascontiguousarray(query).astype(BF16)
    y16 = np.ascontiguousarray(key_value).astype(BF16)
    consts = _prep_consts(Wq, bq, Wk, bk, Wv, bv, Wo, bo, gamma, beta)
    # bo folded into the residual ("query" below is query + bo)
    q32 = np.asarray(query, np.float32) + np.asarray(bo, np.float32)
    in_maps = []
    for c in range(n_cores):
        sl = slice(c * rows, (c + 1) * rows)
        in_maps.append({
            "query": q32[sl],
            "x16": x16[sl],
            "y16": y16[sl],
            **consts,
        })
    return in_maps


_exec_cache = {}


def get_sharded_executor(nc, n_cores=N_CORES):
    """One jitted shard_map callable dispatching the identical per-core
    program to all n_cores devices in a single PJRT execution (vs 8
    serialized per-device dispatches)."""
    key = (id(nc), n_cores)
    if key in _exec_cache:
        return _exec_cache[key]
    import jax
    from jax.sharding import Mesh, PartitionSpec
    from jax.experimental.shard_map import shard_map
    from concourse.bass2jax import (_bass_exec_p, install_neuronx_cc_hook,
                                    partition_id_tensor)

    install_neuronx_cc_hook()
    partition_name = (nc.partition_id_tensor.name
                      if nc.partition_id_tensor else None)
    in_names, out_names, out_avals = [], [], []
    for alloc in nc.m.functions[0].allocations:
        if not isinstance(alloc, mybir.MemoryLocationSet):
            continue
        name = alloc.memorylocations[0].name
        if alloc.kind == "ExternalInput":
            if name != partition_name:
                in_names.append(name)
        elif alloc.kind == "ExternalOutput":
            out_names.append(name)
            out_avals.append(jax.core.ShapedArray(
                tuple(alloc.tensor_shape), mybir.dt.np(alloc.dtype)))
    n_params = len(in_names)
    all_names = list(in_names) + list(out_names)
    if partition_name is not None:
        all_names.append(partition_name)

    def _body(*args):
        operands = list(args)
        if partition_name is not None:
            operands.append(partition_id_tensor())
        return tuple(_bass_exec_p.bind(
            *operands,
            out_avals=tuple(out_avals),
            in_names=tuple(all_names),
            out_names=tuple(out_names),
            lowering_input_output_aliases=(),
            sim_require_finite=True,
            sim_require_nnan=True,
            nc=nc,
        ))

    devices = jax.devices()[:n_cores]
    mesh = Mesh(np.asarray(devices), ("core",))
    in_specs = (PartitionSpec("core"),) * (n_params + len(out_names))
    out_specs = (PartitionSpec("core"),) * len(out_names)
    fn = jax.jit(shard_map(_body, mesh=mesh, in_specs=in_specs,
                           out_specs=out_specs, check_rep=False),
                 keep_unused=True)
    res = (fn, in_names, out_names, out_avals, mesh)
    _exec_cache[key] = res
    return res


def make_global_inputs(query, key_value, Wq, bq, Wk, bk, Wv, bv, Wo, bo,
                       gamma, beta, n_cores=N_CORES):
    """Full-batch inputs + per-core constants tiled along axis 0 so a
    PartitionSpec('core') sharding hands each core the per-core shapes
    the BIR declares."""
    consts = _prep_consts(Wq, bq, Wk, bk, Wv, bv, Wo, bo, gamma, beta)
    g = {k: np.tile(v, (n_cores,) + (1,) * (v.ndim - 1))
         for k, v in consts.items()}
    # bo folded into the residual ("query" below is query + bo)
    g["query"] = np.asarray(query, np.float32) + np.asarray(bo, np.float32)
    g["x16"] = np.ascontiguousarray(query).astype(BF16)
    g["y16"] = np.ascontiguousarray(key_value).astype(BF16)
    return g


def _run_sharded(nc, global_inputs, n_cores=N_CORES):
    fn, in_names, out_names, out_avals, _ = get_sharded_executor(nc, n_cores)
    args = [np.asarray(global_inputs[n]) for n in in_names]
    args += [np.zeros((n_cores * a.shape[0],) + tuple(a.shape[1:]), a.dtype)
             for a in out_avals]
    outs = fn(*args)
    return {n: np.asarray(outs[i]) for i, n in enumerate(out_names)}


def kernel(**inputs) -> np.ndarray:
    nc = _get_nc(RPC)
    try:
        return _run_sharded(nc, make_global_inputs(**inputs))["out"]
    except Exception:
        in_maps = make_in_maps(**inputs)
        results = run_bass_kernel_spmd(nc, in_maps,
                                       list(range(N_CORES))).results
        return np.concatenate([r["out"] for r in results], axis=0)

